# revision 36
# baseline (speedup 1.0000x reference)
"""Multi-head self-attention (RoPE, causal) Trainium2 Bass kernel.

Sharding: head-parallel across 8 NeuronCores. Core c owns heads {2c, 2c+1}
for both batch rows. Each core computes its heads' QKV projection, RoPE,
causal flash attention (scores kept transposed [k, q]), softmax
normalization, and a partial output projection against its 128 columns of
W_o. The host sums the 8 partial projections (the "all-reduce").

v3: every PE matmul runs in (128,128) tile mode so the PE never drains for
a mode switch and LDWEIGHTS always pipelines: scores contract K=128 against
zero-padded K tiles (KAZ0/KAZ1), AV fuses the softmax denominator as a 65th
lhsT column (ones, written by strided memset into the transpose tiles), and
the reciprocal broadcast matmul is K/M-padded. Contiguous DMA layouts,
j-outer QKV overlapped with the first scores group, fast reciprocal, PE
warmup, producer/consumer slab interleave.

Self-contained: hardcodes B=2, S=2048, D=1024, H=16, d_k=64.
"""
import numpy as np
import ml_dtypes

B, S, D, H, DK = 2, 2048, 1024, 16, 64
NCORES = 8
THETA = 10000.0
BS = B * S                   # 4096 flattened tokens (b-major)
KT = D // 128                # 8 contraction tiles
P = 128

bf16 = ml_dtypes.bfloat16

_CACHED_NC = None


def _host_prep(x, token_positions, W_qkv, W_o):
    """Build per-core DRAM input dicts (numpy, bf16), contiguous layouts."""
    cast = lambda a: np.ascontiguousarray(a).astype(bf16)
    X2 = np.asarray(x, np.float32).reshape(BS, D)
    # xt[p, j, kt, s] = X2[512j+s, 128kt+p]  -> per-partition contiguous 8KB
    xt = cast(X2.T.reshape(KT, P, 8, 512).transpose(1, 2, 0, 3))

    pos = np.asarray(token_positions, np.float64)
    inv = THETA ** (-np.arange(0, DK, 2, dtype=np.float64) / DK)   # [32]
    ang = pos[:, None] * inv[None, :]                              # [S, 32]
    cosv = np.cos(ang).T.astype(np.float32)                        # [32, S]
    sinv = np.sin(ang).T.astype(np.float32)
    COS = cast(np.tile(cosv, (4, 1)))                              # [128, S]
    SINS = cast(np.concatenate([-sinv, sinv, -sinv, sinv], 0))     # [128, S]

    perm = np.concatenate([np.arange(0, 64, 2), np.arange(1, 64, 2)])
    tri = cast(np.triu(np.ones((P, P), np.float32)))               # [k,q]: q>=k

    # sel4p[u, g, m] = 1 iff u == g and m < 64 (K=128/M=65 padded broadcast)
    sel = np.zeros((P, 4, 65), np.float32)
    for g in range(4):
        sel[g, g, 0:64] = 1.0

    def warrange(Wrows):                   # [128 rows, D] -> [p, kt, m]
        return cast(Wrows.T.reshape(KT, P, P).transpose(1, 0, 2))

    Wqkv = np.asarray(W_qkv, np.float32)
    Wo = np.asarray(W_o, np.float32)
    maps = []
    for c in range(NCORES):
        hA = 2 * c
        rows = np.concatenate([(hA + 0) * 64 + perm, (hA + 1) * 64 + perm])
        rows_v = np.concatenate([(hA + 0) * 64 + np.arange(64),
                                 (hA + 1) * 64 + np.arange(64)])
        maps.append({
            "xt": xt,
            "wq": warrange(Wqkv[rows]),
            "wk": warrange(Wqkv[D + rows]),
            "wv": warrange(Wqkv[2 * D + rows_v]),
            "wo": cast(Wo[:, P * c:P * c + P].T),                  # [128, 1024]
            "cos": COS,
            "sin": SINS,
            "tri": tri,
            "sel4p": cast(sel),
        })
    return maps


def _build_nc(debug=False):
    """Trace + compile the per-core Bass module (same program on all cores)."""
    from contextlib import ExitStack
    import concourse.bacc as bacc
    import concourse.mybir as mybir
    import concourse.tile as tile
    from concourse.bass import ts

    f32 = mybir.dt.float32
    bf = mybir.dt.bfloat16
    EXP = mybir.ActivationFunctionType.Exp

    nc = bacc.Bacc("TRN2", target_bir_lowering=False, debug=False,
                   enable_asserts=False)

    xt_d = nc.dram_tensor("xt", [P, 8, KT, 512], bf, kind="ExternalInput").ap()
    wq_d = nc.dram_tensor("wq", [P, KT, P], bf, kind="ExternalInput").ap()
    wk_d = nc.dram_tensor("wk", [P, KT, P], bf, kind="ExternalInput").ap()
    wv_d = nc.dram_tensor("wv", [P, KT, P], bf, kind="ExternalInput").ap()
    wo_d = nc.dram_tensor("wo", [P, D], bf, kind="ExternalInput").ap()
    cos_d = nc.dram_tensor("cos", [P, S], bf, kind="ExternalInput").ap()
    sin_d = nc.dram_tensor("sin", [P, S], bf, kind="ExternalInput").ap()
    tri_d = nc.dram_tensor("tri", [P, P], bf, kind="ExternalInput").ap()
    sel_d = nc.dram_tensor("sel4p", [P, 4, 65], bf, kind="ExternalInput").ap()
    yt_d = nc.dram_tensor("yt", [8, P, 8, 512], bf, kind="ExternalOutput").ap()
    if debug:
        dbg = {n: nc.dram_tensor(n, shp, dt, kind="ExternalOutput").ap()
               for n, shp, dt in [
                   ("d_qa", [P, BS], bf), ("d_ka", [P, BS], bf),
                   ("d_vt", [P, BS], bf),
                   ("d_oacc0", [65, 8, 512], bf),
                   ("d_opr", [8, P, 512], bf), ("d_pt", [4, P, 1024], bf)]}

    with tile.TileContext(nc) as tc, ExitStack() as ctx:
        # ---- kernel-lifetime pools ----
        pp = ctx.enter_context(tc.tile_pool(name="persist", bufs=1))
        WO = pp.tile([P, D], bf, tag="wo")
        TRI = pp.tile([P, P], bf, tag="tri")
        SEL = pp.tile([P, 4, 65], bf, tag="sel")
        WARM = pp.tile([P, 512], bf, tag="warm")
        QA = pp.tile([P, BS], bf, tag="qa")
        KAZ = [pp.tile([P, BS], bf, tag=f"kaz{h}", name=f"kaz{h}")
               for h in range(2)]
        VT = pp.tile([P, BS], bf, tag="vt")
        OACC = [pp.tile([65, 8, 512], bf, tag=f"oacc{b}", name=f"oacc{b}")
                for b in range(B)]
        OPR = [pp.tile([P, 512], bf, tag=f"opr{jj}", name=f"opr{jj}")
               for jj in range(8)]
        # bf16 K=128-padded reciprocal pages: rows 0-3 live, rest zero
        RECP = [pp.tile([P, 512], bf, tag=f"recp{q4}", name=f"recp{q4}")
                for q4 in range(4)]
        drp = ctx.enter_context(tc.tile_pool(name="denr", bufs=2))
        vbp = ctx.enter_context(tc.tile_pool(name="vbig", bufs=4))
        vap = ctx.enter_context(tc.tile_pool(name="vaug", bufs=16))
        ybp = ctx.enter_context(tc.tile_pool(name="ybig", bufs=2))
        stg = ctx.enter_context(tc.tile_pool(name="stage", bufs=3))
        ptb = ctx.enter_context(tc.tile_pool(name="ptbig", bufs=16))
        scps = ctx.enter_context(tc.tile_pool(name="scps", bufs=2,
                                              space="PSUM"))

        # ---- PE warmup (HAM) + ACT exp-table preload + zero pads ----
        nc.vector.memset(WARM[:], 0.0)
        for q4 in range(4):
            nc.gpsimd.memset(RECP[q4][:], 0.0)
        nc.vector.memset(KAZ[0][64:128, :], 0.0)
        nc.gpsimd.memset(KAZ[1][0:64, :], 0.0)
        scratch = pp.tile([1, 8], bf, tag="scratch")
        nc.scalar.activation(scratch[:], WARM[0:1, 0:8], EXP, scale=1.0)



        pts = {}
        vas = {}
        vab = {}

        def transposes(b):
            VA = vbp.tile([P, 16, 64], bf, tag="vb", name=f"va{b}")
            VB = vbp.tile([P, 16, 64], bf, tag="vb", name=f"vb{b}")
            for h, VX in ((0, VA), (1, VB)):
                nc.sync.dma_start_transpose(
                    VX[:], VT[64 * h:64 * h + 64, b * S:(b + 1) * S])
            vab[b] = (VA, VB)

        def scores_slab(b, ilist, pool, width):
            """K=128 zero-padded score matmuls + exp for i in ilist."""
            for i in ilist:
                qs_i = 512 * (i // 4)
                qext = S - qs_i
                blk = b * S + 128 * i
                for h in range(2):
                    pt = pool.tile([P, width], bf, tag="pt",
                                   name=f"pt{b}_{i}_{h}")
                    pts[(b, i, h)] = pt
                    off = 0
                    while off < qext:
                        w = min(1024, qext - off)
                        ps = scps.tile([P, 1024], f32, tag="sc", name="ps")
                        vf = max(0, 128 * i - (qs_i + off))
                        for qc in range(0, w, 512):
                            sub = min(512, w - qc)
                            q0 = qs_i + off + qc
                            if q0 + sub <= 128 * i:
                                continue  # fully masked chunk
                            nc.tensor.matmul(
                                ps[:, qc:qc + sub],
                                lhsT=KAZ[h][:, blk:blk + 128],
                                rhs=QA[:, b * S + q0:b * S + q0 + sub],
                                start=True, stop=True)
                        if vf < w:
                            nc.scalar.activation(pt[:, off + vf:off + w],
                                                 ps[:, vf:w], EXP, scale=0.125)
                        if vf > 0:
                            nc.gpsimd.memset(pt[:, off:off + vf], 0.0)
                        off += w
                    dc = 128 * i - qs_i
                    nc.gpsimd.tensor_mul(pt[:, dc:dc + 128],
                                         pt[:, dc:dc + 128], TRI[:])
                # [A(64) | 1 | B(64) | 1] per-head AV lhsT with ones col
                va = vap.tile([P, 132], bf, tag="va", name=f"va{b}_{i}")
                nc.vector.tensor_copy(va[:, 0:64], vab[b][0][:, i, :])
                nc.vector.memset(va[:, 64:65], 1.0)
                nc.gpsimd.tensor_copy(va[:, 65:129], vab[b][1][:, i, :])
                nc.gpsimd.memset(va[:, 129:130], 1.0)
                vas[(b, i)] = va

        # ---- QKV projection, j-outer, interleaved with b0/g0 scores ----
        with tc.tile_pool(name="xtp", bufs=2) as xtp, \
             tc.tile_pool(name="wp", bufs=1) as wp, \
             tc.tile_pool(name="swap", bufs=2) as swp, \
             tc.tile_pool(name="qkvps", bufs=2, space="PSUM") as qps:
            WQ = wp.tile([P, KT, P], bf, tag="wq")
            WK = wp.tile([P, KT, P], bf, tag="wk")
            WV = wp.tile([P, KT, P], bf, tag="wv")
            COS = wp.tile([P, S], bf, tag="cos")
            SIN = wp.tile([P, S], bf, tag="sin")
            nc.sync.dma_start(WQ[:], wq_d)
            nc.sync.dma_start(WK[:], wk_d)
            nc.sync.dma_start(WV[:], wv_d)

            # HAM warmup: garbage matmuls while xt chunk 0 streams in
            for wi in range(10):
                pw = qps.tile([P, 512], f32, tag="qkv", name="pw")
                nc.tensor.matmul(pw[:], lhsT=WARM[:, 0:128], rhs=WARM[:],
                                 start=True, stop=True)

            def rope_one(A, lo, hi, b, SW):
                """RoPE on A rows [lo,hi) for batch b (paired 32-blocks)."""
                bsl = ts(b, S)
                for blkk in range(lo // 32, hi // 32):
                    src = blkk ^ 1
                    nc.gpsimd.dma_start(SW[32 * blkk:32 * blkk + 32, :],
                                        A[32 * src:32 * src + 32, bsl])
                for ch in range(2):
                    asl = slice(b * S + 1024 * ch, b * S + 1024 * ch + 1024)
                    csl = ts(ch, 1024)
                    nc.vector.tensor_mul(A[lo:hi, asl], A[lo:hi, asl],
                                         COS[lo:hi, csl])
                    nc.vector.tensor_mul(SW[lo:hi, csl], SW[lo:hi, csl],
                                         SIN[lo:hi, csl])
                    nc.vector.tensor_add(A[lo:hi, asl], A[lo:hi, asl],
                                         SW[lo:hi, csl])

            def rope(b):
                SWQ = swp.tile([P, S], bf, tag="sw", name="swq")
                rope_one(QA, 0, 128, b, SWQ)
                SWK = swp.tile([P, S], bf, tag="sw", name="swk")
                rope_one(KAZ[0], 0, 64, b, SWK)
                rope_one(KAZ[1], 64, 128, b, SWK)

            xts = {}

            def fetch(j):
                t = xtp.tile([P, KT, 512], bf, tag="xt", name=f"xt{j}")
                nc.sync.dma_start(t[:], xt_d[:, j])
                xts[j] = t

            fetch(0)
            fetch(1)
            nc.sync.dma_start(COS[:], cos_d)
            nc.sync.dma_start(SIN[:], sin_d)
            nc.sync.dma_start(WO[:], wo_d)
            nc.sync.dma_start(TRI[:], tri_d)
            nc.sync.dma_start(SEL[:], sel_d)
            for j in range(2, 8):
                fetch(j)     # xtp rotation semaphores pace these
            for j in range(8):
                for kind, W in (("q", WQ), ("k", WK), ("v", WV)):
                    ps = qps.tile([P, 512], f32, tag="qkv", name="ps")
                    for kt in range(KT):
                        nc.tensor.matmul(ps[:], lhsT=W[:, kt, :],
                                         rhs=xts[j][:, kt, :],
                                         start=(kt == 0), stop=(kt == KT - 1))
                    if kind == "q":
                        nc.scalar.copy(QA[:, ts(j, 512)], ps[:])
                    elif kind == "v":
                        nc.vector.tensor_copy(VT[:, ts(j, 512)], ps[:])
                    else:
                        nc.scalar.copy(KAZ[0][0:64, ts(j, 512)], ps[0:64, :])
                        nc.vector.tensor_copy(KAZ[1][64:128, ts(j, 512)],
                                              ps[64:128, :])
                del xts[j]
                if j == 3:
                    rope(0)
                    transposes(0)
                if j >= 4:       # overlap first scores group with QKV
                    scores_slab(0, [2 * (j - 4), 2 * (j - 4) + 1], ptb, 2048)
            rope(1)

        # ---- attention consumers + remaining scores groups ----
        pts_small = ctx.enter_context(tc.tile_pool(name="ptsm", bufs=16))
        pay = ctx.enter_context(tc.tile_pool(name="pay", bufs=3, space="PSUM"))

        def ilist_for(g, j):
            return [i for i in range(8 * g, 8 * g + 8) if i <= 4 * j + 3]

        def av(b, g, j):
            """Fused AV + denominator (65th ones column), M=65 matmuls."""
            ilist = ilist_for(g, j)
            if not ilist:
                return
            for h in range(2):
                pa = pay.tile([P, 512], f32, tag="pay", name="pa")
                for n, i in enumerate(ilist):
                    qs_i = 512 * (i // 4)
                    o0 = 512 * j - qs_i
                    nc.tensor.matmul(
                        pa[0:65, :], lhsT=vas[(b, i)][:, 65 * h:65 * h + 65],
                        rhs=pts[(b, i, h)][:, o0:o0 + 512],
                        start=(n == 0), stop=(n == len(ilist) - 1))
                u8 = j * 2 + h
                if g == 0:
                    if h == 0:
                        nc.vector.tensor_copy(OACC[b][:, u8, :], pa[0:65, :])
                    else:
                        nc.scalar.copy(OACC[b][:, u8, :], pa[0:65, :])
                else:
                    nc.vector.tensor_add(OACC[b][:, u8, :],
                                         OACC[b][:, u8, :], pa[0:65, :])

        def div_prep(b, half):
            """Gather denominators, reciprocal, broadcast page (early)."""
            q4 = b * 2 + half
            u0 = 4 * half
            SUMS = drp.tile([4, 512], bf, tag="sums", name="sums")
            SUMF = drp.tile([4, 512], f32, tag="sumf", name="sumf")
            REC = drp.tile([4, 512], f32, tag="rec", name="rec")
            nc.gpsimd.dma_start(SUMS[:], OACC[b][64:65, u0:u0 + 4, :])
            nc.vector.tensor_copy(SUMF[:], SUMS[:])
            nc.vector.reciprocal_approx_fast(REC[:], SUMF[:])
            nc.vector.tensor_copy(RECP[q4][0:4, :], REC[:])

        def div_oproj(b, half):
            """Padded SEL broadcast, OPR, o_proj, store."""
            q4 = b * 2 + half
            for j in (2 * half, 2 * half + 1):
                jj = b * 4 + j
                for h in range(2):
                    u4 = (j % 2) * 2 + h
                    u8 = j * 2 + h
                    pb = pay.tile([P, 512], f32, tag="pay", name="pb")
                    nc.tensor.matmul(pb[0:65, :], lhsT=SEL[:, u4, :],
                                     rhs=RECP[q4][:], start=True, stop=True)
                    if h == 0:
                        nc.vector.tensor_mul(OPR[jj][0:64, :],
                                             OACC[b][0:64, u8, :],
                                             pb[0:64, :])
                    else:
                        tb = stg.tile([64, 512], bf, tag="tb", name="tb")
                        nc.vector.tensor_mul(tb[:], OACC[b][0:64, u8, :],
                                             pb[0:64, :])
                        nc.gpsimd.dma_start(OPR[jj][64:128, :], tb[:])
                yb = ybp.tile([P, 8, 512], bf, tag="yb", name="yb")
                for et in range(8):
                    py = pay.tile([P, 512], f32, tag="pay", name="py")
                    nc.tensor.matmul(py[:], lhsT=WO[:, ts(et, P)],
                                     rhs=OPR[jj][:], start=True, stop=True)
                    if et % 2 == 1:
                        nc.scalar.copy(yb[:, et, :], py[:])
                    else:
                        nc.vector.tensor_copy(yb[:, et, :], py[:])
                nc.sync.dma_start(yt_d[jj], yb[:])

        # interleave: [producer slab | consumer chunk] ...
        def P_slabs(b, g, pool, width):
            i0 = 8 * g
            for k in range(4):
                yield lambda b=b, sl=[i0 + 2 * k, i0 + 2 * k + 1], p=pool, \
                    w=width: scores_slab(b, sl, p, w)

        def C_chunks(b):
            yield lambda: av(b, 0, 0)
            yield lambda: av(b, 0, 1)
            yield lambda: div_prep(b, 0)
            yield lambda: div_oproj(b, 0)
            yield lambda: av(b, 0, 2)
            yield lambda: av(b, 0, 3)
            yield lambda: av(b, 1, 2)
            yield lambda: av(b, 1, 3)
            yield lambda: div_prep(b, 1)
            yield lambda: div_oproj(b, 1)

        transposes(1)
        producers = list(P_slabs(0, 1, pts_small, 1024)) + \
            list(P_slabs(1, 0, ptb, 2048)) + list(P_slabs(1, 1, pts_small,
                                                          1024))
        consumers = list(C_chunks(0)) + list(C_chunks(1))
        pi, ci = 0, 0
        # 12 producers, 20 consumers; front-load producers, hide div latency
        order = ["P", "C", "P", "C", "P", "C", "C", "P", "C", "C",
                 "P", "C", "P", "C", "P", "C", "P", "C",
                 "P", "C", "P", "C", "P", "C", "C", "P", "C", "C",
                 "C", "C", "C", "C"]
        for kind in order:
            if kind == "P" and pi < len(producers):
                producers[pi]()
                pi += 1
            elif kind == "C" and ci < len(consumers):
                consumers[ci]()
                ci += 1
        while pi < len(producers):
            producers[pi]()
            pi += 1
        while ci < len(consumers):
            consumers[ci]()
            ci += 1

        if debug:
            nc.sync.dma_start(dbg["d_qa"], QA[:])
            ka = pp.tile([P, BS], bf, tag="kadbg")
            nc.vector.tensor_copy(ka[0:64, :], KAZ[0][0:64, :])
            nc.vector.tensor_copy(ka[64:128, :], KAZ[1][64:128, :])
            nc.sync.dma_start(dbg["d_ka"], ka[:])
            nc.sync.dma_start(dbg["d_vt"], VT[:])
            nc.sync.dma_start(dbg["d_oacc0"], OACC[0][:])
            for jj in range(8):
                nc.sync.dma_start(dbg["d_opr"][jj], OPR[jj][:])
            nc.sync.dma_start(dbg["d_pt"][0], pts[(1, 8, 0)][:, 0:1024])
            nc.sync.dma_start(dbg["d_pt"][1], pts[(1, 8, 1)][:, 0:1024])
            nc.sync.dma_start(dbg["d_pt"][2][:, 0:512],
                              pts[(1, 12, 0)][:, 0:512])
            nc.sync.dma_start(dbg["d_pt"][3][:, 0:512],
                              pts[(1, 12, 1)][:, 0:512])

    nc.compile()
    return nc


def get_nc():
    global _CACHED_NC
    if _CACHED_NC is None:
        _CACHED_NC = _build_nc()
    return _CACHED_NC


def run_on_hw(in_maps, **kwargs):
    from concourse.bass_utils import run_bass_kernel_spmd
    nc = get_nc()
    return run_bass_kernel_spmd(nc, in_maps, core_ids=list(range(NCORES)),
                                **kwargs)


def gather(results):
    acc = np.zeros((D, BS), np.float32)
    for r in results:
        yt = np.asarray(r["yt"]).astype(np.float32)    # [8, 128, 8, 512]
        acc += yt.transpose(2, 1, 0, 3).reshape(D, BS)
    return np.ascontiguousarray(acc.T).reshape(B, S, D).astype(np.float32)


def kernel(x, token_positions, W_qkv, W_o):
    in_maps = _host_prep(x, token_positions, W_qkv, W_o)
    res = run_on_hw(in_maps)
    return gather(res.results)


# revision 37
# speedup vs baseline: 1.0692x; 1.0692x over previous
"""Multi-head self-attention (RoPE, causal) Trainium2 Bass kernel.

Sharding: head-parallel across 8 NeuronCores. Core c owns heads {2c, 2c+1}
for both batch rows. Each core computes its heads' QKV projection, RoPE,
causal flash attention (scores kept transposed [k, q]), softmax
normalization, and a partial output projection against its 128 columns of
W_o. The host sums the 8 partial projections (the "all-reduce").

v3: every PE matmul runs in (128,128) tile mode so the PE never drains for
a mode switch and LDWEIGHTS always pipelines: scores contract K=128 against
zero-padded K tiles (KAZ0/KAZ1), AV fuses the softmax denominator as a 65th
lhsT column (ones, written by strided memset into the transpose tiles), and
the reciprocal broadcast matmul is K/M-padded. Contiguous DMA layouts,
j-outer QKV overlapped with the first scores group, fast reciprocal, PE
warmup, producer/consumer slab interleave.

Self-contained: hardcodes B=2, S=2048, D=1024, H=16, d_k=64.
"""
import numpy as np
import ml_dtypes

B, S, D, H, DK = 2, 2048, 1024, 16, 64
NCORES = 8
THETA = 10000.0
BS = B * S                   # 4096 flattened tokens (b-major)
KT = D // 128                # 8 contraction tiles
P = 128

bf16 = ml_dtypes.bfloat16

_CACHED_NC = None


def _host_prep(x, token_positions, W_qkv, W_o):
    """Build per-core DRAM input dicts (numpy, bf16), contiguous layouts."""
    cast = lambda a: np.ascontiguousarray(a).astype(bf16)
    X2 = np.asarray(x, np.float32).reshape(BS, D)
    # xt[p, j, kt, s] = X2[512j+s, 128kt+p]  -> per-partition contiguous 8KB
    xt = cast(X2.T.reshape(KT, P, 8, 512).transpose(1, 2, 0, 3))

    pos = np.asarray(token_positions, np.float64)
    inv = THETA ** (-np.arange(0, DK, 2, dtype=np.float64) / DK)   # [32]
    ang = pos[:, None] * inv[None, :]                              # [S, 32]
    cosv = np.cos(ang).T.astype(np.float32)                        # [32, S]
    sinv = np.sin(ang).T.astype(np.float32)
    COS = cast(np.tile(cosv, (4, 1)))                              # [128, S]
    SINS = cast(np.concatenate([-sinv, sinv, -sinv, sinv], 0))     # [128, S]

    perm = np.concatenate([np.arange(0, 64, 2), np.arange(1, 64, 2)])
    tri = cast(np.triu(np.ones((P, P), np.float32)))               # [k,q]: q>=k

    # sel4p[u, g, m] = 1 iff u == g and m < 64 (K=128/M=65 padded broadcast)
    sel = np.zeros((P, 4, 65), np.float32)
    for g in range(4):
        sel[g, g, 0:64] = 1.0

    def warrange(Wrows):                   # [128 rows, D] -> [p, kt, m]
        return cast(Wrows.T.reshape(KT, P, P).transpose(1, 0, 2))

    Wqkv = np.asarray(W_qkv, np.float32)
    Wo = np.asarray(W_o, np.float32)
    maps = []
    for c in range(NCORES):
        hA = 2 * c
        rows = np.concatenate([(hA + 0) * 64 + perm, (hA + 1) * 64 + perm])
        rows_v = np.concatenate([(hA + 0) * 64 + np.arange(64),
                                 (hA + 1) * 64 + np.arange(64)])
        maps.append({
            "xt": xt,
            "wq": warrange(Wqkv[rows]),
            "wk": warrange(Wqkv[D + rows]),
            "wv": warrange(Wqkv[2 * D + rows_v]),
            "wo": cast(Wo[:, P * c:P * c + P].T),                  # [128, 1024]
            "cos": COS,
            "sin": SINS,
            "tri": tri,
            "sel4p": cast(sel),
        })
    return maps


def _build_nc(debug=False):
    """Trace + compile the per-core Bass module (same program on all cores)."""
    from contextlib import ExitStack
    import concourse.bacc as bacc
    import concourse.mybir as mybir
    import concourse.tile as tile
    from concourse.bass import ts

    f32 = mybir.dt.float32
    bf = mybir.dt.bfloat16
    EXP = mybir.ActivationFunctionType.Exp

    nc = bacc.Bacc("TRN2", target_bir_lowering=False, debug=False,
                   enable_asserts=False)

    xt_d = nc.dram_tensor("xt", [P, 8, KT, 512], bf, kind="ExternalInput").ap()
    wq_d = nc.dram_tensor("wq", [P, KT, P], bf, kind="ExternalInput").ap()
    wk_d = nc.dram_tensor("wk", [P, KT, P], bf, kind="ExternalInput").ap()
    wv_d = nc.dram_tensor("wv", [P, KT, P], bf, kind="ExternalInput").ap()
    wo_d = nc.dram_tensor("wo", [P, D], bf, kind="ExternalInput").ap()
    cos_d = nc.dram_tensor("cos", [P, S], bf, kind="ExternalInput").ap()
    sin_d = nc.dram_tensor("sin", [P, S], bf, kind="ExternalInput").ap()
    tri_d = nc.dram_tensor("tri", [P, P], bf, kind="ExternalInput").ap()
    sel_d = nc.dram_tensor("sel4p", [P, 4, 65], bf, kind="ExternalInput").ap()
    yt_d = nc.dram_tensor("yt", [8, P, 8, 512], bf, kind="ExternalOutput").ap()
    if debug:
        dbg = {n: nc.dram_tensor(n, shp, dt, kind="ExternalOutput").ap()
               for n, shp, dt in [
                   ("d_qa", [P, BS], bf), ("d_ka", [P, BS], bf),
                   ("d_vt", [P, BS], bf),
                   ("d_oacc0", [65, 8, 512], bf),
                   ("d_opr", [8, P, 512], bf), ("d_pt", [4, P, 1024], bf)]}

    with tile.TileContext(nc) as tc, ExitStack() as ctx:
        # ---- kernel-lifetime pools ----
        pp = ctx.enter_context(tc.tile_pool(name="persist", bufs=1))
        WO = pp.tile([P, D], bf, tag="wo")
        TRI = pp.tile([P, P], bf, tag="tri")
        SEL = pp.tile([P, 4, 65], bf, tag="sel")
        WARM = pp.tile([P, 512], bf, tag="warm")
        QA = pp.tile([P, BS], bf, tag="qa")
        KAZ = [pp.tile([P, BS], bf, tag=f"kaz{h}", name=f"kaz{h}")
               for h in range(2)]
        VT = pp.tile([P, BS], bf, tag="vt")
        OACC = [pp.tile([65, 8, 512], bf, tag=f"oacc{b}", name=f"oacc{b}")
                for b in range(B)]
        OPR = [pp.tile([P, 512], bf, tag=f"opr{jj}", name=f"opr{jj}")
               for jj in range(8)]
        # bf16 K=128-padded reciprocal pages: rows 0-3 live, rest zero
        RECP = [pp.tile([P, 512], bf, tag=f"recp{q4}", name=f"recp{q4}")
                for q4 in range(4)]
        drp = ctx.enter_context(tc.tile_pool(name="denr", bufs=2))
        vbp = ctx.enter_context(tc.tile_pool(name="vbig", bufs=4))
        vap = ctx.enter_context(tc.tile_pool(name="vaug", bufs=16))
        ybp = ctx.enter_context(tc.tile_pool(name="ybig", bufs=2))
        stg = ctx.enter_context(tc.tile_pool(name="stage", bufs=3))
        ptb = ctx.enter_context(tc.tile_pool(name="ptbig", bufs=16))
        scps = ctx.enter_context(tc.tile_pool(name="scps", bufs=3,
                                              space="PSUM"))

        # ---- PE warmup (HAM) + ACT exp-table preload + zero pads ----
        nc.vector.memset(WARM[:], 0.0)
        for q4 in range(4):
            nc.gpsimd.memset(RECP[q4][:], 0.0)
        nc.vector.memset(KAZ[0][64:128, :], 0.0)
        nc.gpsimd.memset(KAZ[1][0:64, :], 0.0)
        scratch = pp.tile([1, 8], bf, tag="scratch")
        nc.scalar.activation(scratch[:], WARM[0:1, 0:8], EXP, scale=1.0)



        pts = {}
        vas = {}
        vab = {}

        def transposes(b):
            VA = vbp.tile([P, 16, 64], bf, tag="vb", name=f"va{b}")
            VB = vbp.tile([P, 16, 64], bf, tag="vb", name=f"vb{b}")
            for h, VX in ((0, VA), (1, VB)):
                nc.sync.dma_start_transpose(
                    VX[:], VT[64 * h:64 * h + 64, b * S:(b + 1) * S])
            vab[b] = (VA, VB)

        def scores_slab(b, ilist, pool, width):
            """K=128 zero-padded score matmuls + exp for i in ilist."""
            for i in ilist:
                qs_i = 512 * (i // 4)
                qext = S - qs_i
                blk = b * S + 128 * i
                for h in range(2):
                    pt = pool.tile([P, width], bf, tag="pt",
                                   name=f"pt{b}_{i}_{h}")
                    pts[(b, i, h)] = pt
                    off = 0
                    while off < qext:
                        w = min(1024, qext - off)
                        ps = scps.tile([P, 1024], f32, tag="sc", name="ps")
                        vf = max(0, 128 * i - (qs_i + off))
                        for qc in range(0, w, 512):
                            sub = min(512, w - qc)
                            q0 = qs_i + off + qc
                            if q0 + sub <= 128 * i:
                                continue  # fully masked chunk
                            nc.tensor.matmul(
                                ps[:, qc:qc + sub],
                                lhsT=KAZ[h][:, blk:blk + 128],
                                rhs=QA[:, b * S + q0:b * S + q0 + sub],
                                start=True, stop=True)
                        if vf < w:
                            nc.scalar.activation(pt[:, off + vf:off + w],
                                                 ps[:, vf:w], EXP, scale=0.125)
                        if vf > 0:
                            nc.gpsimd.memset(pt[:, off:off + vf], 0.0)
                        off += w
                    dc = 128 * i - qs_i
                    nc.gpsimd.tensor_mul(pt[:, dc:dc + 128],
                                         pt[:, dc:dc + 128], TRI[:])
                # [A(64) | 1 | B(64) | 1] per-head AV lhsT with ones col
                va = vap.tile([P, 132], bf, tag="va", name=f"va{b}_{i}")
                nc.vector.tensor_copy(va[:, 0:64], vab[b][0][:, i, :])
                nc.vector.memset(va[:, 64:65], 1.0)
                nc.gpsimd.tensor_copy(va[:, 65:129], vab[b][1][:, i, :])
                nc.gpsimd.memset(va[:, 129:130], 1.0)
                vas[(b, i)] = va

        # ---- QKV projection, j-outer, interleaved with b0/g0 scores ----
        with tc.tile_pool(name="xtp", bufs=2) as xtp, \
             tc.tile_pool(name="wp", bufs=1) as wp, \
             tc.tile_pool(name="swap", bufs=2) as swp, \
             tc.tile_pool(name="qkvps", bufs=2, space="PSUM") as qps:
            WQ = wp.tile([P, KT, P], bf, tag="wq")
            WK = wp.tile([P, KT, P], bf, tag="wk")
            WV = wp.tile([P, KT, P], bf, tag="wv")
            COS = wp.tile([P, S], bf, tag="cos")
            SIN = wp.tile([P, S], bf, tag="sin")
            nc.sync.dma_start(WQ[:], wq_d)
            nc.sync.dma_start(WK[:], wk_d)
            nc.sync.dma_start(WV[:], wv_d)

            # HAM warmup: garbage matmuls while xt chunk 0 streams in
            for wi in range(10):
                pw = qps.tile([P, 512], f32, tag="qkv", name="pw")
                nc.tensor.matmul(pw[:], lhsT=WARM[:, 0:128], rhs=WARM[:],
                                 start=True, stop=True)

            def rope_one(A, lo, hi, b, SW):
                """RoPE on A rows [lo,hi) for batch b (paired 32-blocks)."""
                bsl = ts(b, S)
                for blkk in range(lo // 32, hi // 32):
                    src = blkk ^ 1
                    nc.gpsimd.dma_start(SW[32 * blkk:32 * blkk + 32, :],
                                        A[32 * src:32 * src + 32, bsl])
                for ch in range(2):
                    asl = slice(b * S + 1024 * ch, b * S + 1024 * ch + 1024)
                    csl = ts(ch, 1024)
                    nc.vector.tensor_mul(A[lo:hi, asl], A[lo:hi, asl],
                                         COS[lo:hi, csl])
                    nc.vector.tensor_mul(SW[lo:hi, csl], SW[lo:hi, csl],
                                         SIN[lo:hi, csl])
                    nc.vector.tensor_add(A[lo:hi, asl], A[lo:hi, asl],
                                         SW[lo:hi, csl])

            def rope(b):
                SWQ = swp.tile([P, S], bf, tag="sw", name="swq")
                rope_one(QA, 0, 128, b, SWQ)
                SWK = swp.tile([P, S], bf, tag="sw", name="swk")
                rope_one(KAZ[0], 0, 64, b, SWK)
                rope_one(KAZ[1], 64, 128, b, SWK)

            xts = {}

            def fetch(j):
                t = xtp.tile([P, KT, 512], bf, tag="xt", name=f"xt{j}")
                nc.sync.dma_start(t[:], xt_d[:, j])
                xts[j] = t

            fetch(0)
            fetch(1)
            nc.sync.dma_start(COS[:], cos_d)
            nc.sync.dma_start(SIN[:], sin_d)
            nc.sync.dma_start(WO[:], wo_d)
            nc.sync.dma_start(TRI[:], tri_d)
            nc.sync.dma_start(SEL[:], sel_d)
            for j in range(2, 8):
                fetch(j)     # xtp rotation semaphores pace these
            for j in range(8):
                for kind, W in (("q", WQ), ("k", WK), ("v", WV)):
                    ps = qps.tile([P, 512], f32, tag="qkv", name="ps")
                    for kt in range(KT):
                        nc.tensor.matmul(ps[:], lhsT=W[:, kt, :],
                                         rhs=xts[j][:, kt, :],
                                         start=(kt == 0), stop=(kt == KT - 1))
                    if kind == "q":
                        nc.scalar.copy(QA[:, ts(j, 512)], ps[:])
                    elif kind == "v":
                        nc.vector.tensor_copy(VT[:, ts(j, 512)], ps[:])
                    else:
                        nc.scalar.copy(KAZ[0][0:64, ts(j, 512)], ps[0:64, :])
                        nc.vector.tensor_copy(KAZ[1][64:128, ts(j, 512)],
                                              ps[64:128, :])
                del xts[j]
                if j == 3:
                    rope(0)
                    transposes(0)
                if j >= 4:       # overlap first scores group with QKV
                    scores_slab(0, [2 * (j - 4), 2 * (j - 4) + 1], ptb, 2048)
            rope(1)

        # ---- attention consumers + remaining scores groups ----
        pts_small = ctx.enter_context(tc.tile_pool(name="ptsm", bufs=16))
        pay = ctx.enter_context(tc.tile_pool(name="pay", bufs=2, space="PSUM"))

        def ilist_for(g, j):
            return [i for i in range(8 * g, 8 * g + 8) if i <= 4 * j + 3]

        def av(b, g, j):
            """Fused AV + denominator (65th ones column), M=65 matmuls."""
            ilist = ilist_for(g, j)
            if not ilist:
                return
            for h in range(2):
                pa = pay.tile([P, 512], f32, tag="pay", name="pa")
                for n, i in enumerate(ilist):
                    qs_i = 512 * (i // 4)
                    o0 = 512 * j - qs_i
                    nc.tensor.matmul(
                        pa[0:65, :], lhsT=vas[(b, i)][:, 65 * h:65 * h + 65],
                        rhs=pts[(b, i, h)][:, o0:o0 + 512],
                        start=(n == 0), stop=(n == len(ilist) - 1))
                u8 = j * 2 + h
                if g == 0:
                    if h == 0:
                        nc.vector.tensor_copy(OACC[b][:, u8, :], pa[0:65, :])
                    else:
                        nc.scalar.copy(OACC[b][:, u8, :], pa[0:65, :])
                else:
                    nc.vector.tensor_add(OACC[b][:, u8, :],
                                         OACC[b][:, u8, :], pa[0:65, :])

        def div_prep(b, half):
            """Gather denominators, reciprocal, broadcast page (early)."""
            q4 = b * 2 + half
            u0 = 4 * half
            SUMS = drp.tile([4, 512], bf, tag="sums", name="sums")
            SUMF = drp.tile([4, 512], f32, tag="sumf", name="sumf")
            REC = drp.tile([4, 512], f32, tag="rec", name="rec")
            nc.gpsimd.dma_start(SUMS[:], OACC[b][64:65, u0:u0 + 4, :])
            nc.vector.tensor_copy(SUMF[:], SUMS[:])
            nc.vector.reciprocal_approx_fast(REC[:], SUMF[:])
            nc.vector.tensor_copy(RECP[q4][0:4, :], REC[:])

        def div_oproj(b, half):
            """Padded SEL broadcast, OPR, o_proj, store."""
            q4 = b * 2 + half
            for j in (2 * half, 2 * half + 1):
                jj = b * 4 + j
                for h in range(2):
                    u4 = (j % 2) * 2 + h
                    u8 = j * 2 + h
                    pb = pay.tile([P, 512], f32, tag="pay", name="pb")
                    nc.tensor.matmul(pb[0:65, :], lhsT=SEL[:, u4, :],
                                     rhs=RECP[q4][:], start=True, stop=True)
                    if h == 0:
                        nc.vector.tensor_mul(OPR[jj][0:64, :],
                                             OACC[b][0:64, u8, :],
                                             pb[0:64, :])
                    else:
                        tb = stg.tile([64, 512], bf, tag="tb", name="tb")
                        nc.vector.tensor_mul(tb[:], OACC[b][0:64, u8, :],
                                             pb[0:64, :])
                        nc.gpsimd.dma_start(OPR[jj][64:128, :], tb[:])
                yb = ybp.tile([P, 8, 512], bf, tag="yb", name="yb")
                for et in range(8):
                    py = pay.tile([P, 512], f32, tag="pay", name="py")
                    nc.tensor.matmul(py[:], lhsT=WO[:, ts(et, P)],
                                     rhs=OPR[jj][:], start=True, stop=True)
                    if et % 2 == 1:
                        nc.scalar.copy(yb[:, et, :], py[:])
                    else:
                        nc.vector.tensor_copy(yb[:, et, :], py[:])
                nc.sync.dma_start(yt_d[jj], yb[:])

        # interleave: [producer slab | consumer chunk] ...
        def P_slabs(b, g, pool, width):
            i0 = 8 * g
            for k in range(4):
                yield lambda b=b, sl=[i0 + 2 * k, i0 + 2 * k + 1], p=pool, \
                    w=width: scores_slab(b, sl, p, w)

        def C_chunks(b):
            yield lambda: av(b, 0, 0)
            yield lambda: av(b, 0, 1)
            yield lambda: div_prep(b, 0)
            yield lambda: div_oproj(b, 0)
            yield lambda: av(b, 0, 2)
            yield lambda: av(b, 0, 3)
            yield lambda: av(b, 1, 2)
            yield lambda: av(b, 1, 3)
            yield lambda: div_prep(b, 1)
            yield lambda: div_oproj(b, 1)

        transposes(1)
        producers = list(P_slabs(0, 1, pts_small, 1024)) + \
            list(P_slabs(1, 0, ptb, 2048)) + list(P_slabs(1, 1, pts_small,
                                                          1024))
        consumers = list(C_chunks(0)) + list(C_chunks(1))
        pi, ci = 0, 0
        # 12 producers, 20 consumers; front-load producers, hide div latency
        order = ["P", "C", "P", "C", "P", "C", "C", "P", "C", "C",
                 "P", "C", "P", "C", "P", "C", "P", "C",
                 "P", "C", "P", "C", "P", "C", "C", "P", "C", "C",
                 "C", "C", "C", "C"]
        for kind in order:
            if kind == "P" and pi < len(producers):
                producers[pi]()
                pi += 1
            elif kind == "C" and ci < len(consumers):
                consumers[ci]()
                ci += 1
        while pi < len(producers):
            producers[pi]()
            pi += 1
        while ci < len(consumers):
            consumers[ci]()
            ci += 1

        if debug:
            nc.sync.dma_start(dbg["d_qa"], QA[:])
            ka = pp.tile([P, BS], bf, tag="kadbg")
            nc.vector.tensor_copy(ka[0:64, :], KAZ[0][0:64, :])
            nc.vector.tensor_copy(ka[64:128, :], KAZ[1][64:128, :])
            nc.sync.dma_start(dbg["d_ka"], ka[:])
            nc.sync.dma_start(dbg["d_vt"], VT[:])
            nc.sync.dma_start(dbg["d_oacc0"], OACC[0][:])
            for jj in range(8):
                nc.sync.dma_start(dbg["d_opr"][jj], OPR[jj][:])
            nc.sync.dma_start(dbg["d_pt"][0], pts[(1, 8, 0)][:, 0:1024])
            nc.sync.dma_start(dbg["d_pt"][1], pts[(1, 8, 1)][:, 0:1024])
            nc.sync.dma_start(dbg["d_pt"][2][:, 0:512],
                              pts[(1, 12, 0)][:, 0:512])
            nc.sync.dma_start(dbg["d_pt"][3][:, 0:512],
                              pts[(1, 12, 1)][:, 0:512])

    nc.compile()
    return nc


def get_nc():
    global _CACHED_NC
    if _CACHED_NC is None:
        _CACHED_NC = _build_nc()
    return _CACHED_NC


def run_on_hw(in_maps, **kwargs):
    from concourse.bass_utils import run_bass_kernel_spmd
    nc = get_nc()
    return run_bass_kernel_spmd(nc, in_maps, core_ids=list(range(NCORES)),
                                **kwargs)


def gather(results):
    acc = np.zeros((D, BS), np.float32)
    for r in results:
        yt = np.asarray(r["yt"]).astype(np.float32)    # [8, 128, 8, 512]
        acc += yt.transpose(2, 1, 0, 3).reshape(D, BS)
    return np.ascontiguousarray(acc.T).reshape(B, S, D).astype(np.float32)


def kernel(x, token_positions, W_qkv, W_o):
    in_maps = _host_prep(x, token_positions, W_qkv, W_o)
    res = run_on_hw(in_maps)
    return gather(res.results)
